# revision 1
# baseline (speedup 1.0000x reference)
"""Trainium2 Bass kernel for nn_MemristorCNN (embedding_lookup, 8 cores).

Strategy (per sharding hint):
- Host gathers the codebook weight W1 = values[w_idx1] and ships the
  *gathered weight* in bf16, column-sharded over in_features (12544
  features = 4 conv2 output channels per core), pre-transposed to
  [12544, 512] so the fc1 moving operand streams as contiguous
  [128, 512] tiles; PSUM accumulation stays fp32.
- Conv stack runs data-parallel (4 images per core); conv1 packs
  (half-image, dy) into K with 3 dx-shift PSUM passes; conv2 packs
  (image, channel, dx-pair) into K=128 with 6 tap passes over a
  twice-replicated (dx-shifted) input.
- AllToAll redistributes conv output h from batch-sharded to
  feature-sharded; PE transposes h to feature-major; fc1 partial
  matmul streams the weight tiles; ReduceScatter sums partials so
  core c ends with images [4c, 4c+4); relu + fc2 finish on device and
  the host concatenates the per-core [4, 4] outputs.
"""

import sys

import numpy as np
import ml_dtypes

BF16NP = ml_dtypes.bfloat16

for _p in ("/opt/trn_rl_repo",):
    if _p not in sys.path:
        sys.path.insert(0, _p)

import concourse.bacc as bacc
import concourse.bass as bass  # noqa: F401
import concourse.tile as tile
from concourse import mybir
from concourse.bass_utils import run_bass_kernel_spmd

F32 = mybir.dt.float32
BF16 = mybir.dt.bfloat16
RELU = mybir.ActivationFunctionType.Relu
COPY = mybir.ActivationFunctionType.Copy

N_CORES = 8
B = 32
IMG = 224
C1, C2 = 16, 32
PH, PW = 112, 112
HH, HW = 56, 56
FEAT = C2 * HH * HW          # 100352
FSH = FEAT // N_CORES        # 12544
NK = FSH // 128              # 98
H1 = 512
NOUT = 4

_CACHE = {}


def _build_program(w_bufs: int, stop_after: str = 'full'):
    nc = bacc.Bacc("TRN2", target_bir_lowering=False, debug=False,
                   num_devices=N_CORES)
    _emit(nc, w_bufs, stop_after)
    nc.compile()
    return nc


def _emit(nc, w_bufs: int, stop_after: str):
    # ---- kernel I/O ----
    x9_t = nc.dram_tensor("x9", [72, PH, 232], BF16, kind="ExternalInput")
    s1_t = nc.dram_tensor("s1", [72, 128], BF16, kind="ExternalInput")
    s2_t = nc.dram_tensor("s2", [6, 128, 128], BF16, kind="ExternalInput")
    w1t_t = nc.dram_tensor("w1t", [FSH, H1], BF16, kind="ExternalInput")
    b1t_t = nc.dram_tensor("b1t", [128, 4, 4], F32, kind="ExternalInput")
    w2t_t = nc.dram_tensor("w2t", [H1, NOUT], F32, kind="ExternalInput")
    b2t_t = nc.dram_tensor("b2t", [4, 4], F32, kind="ExternalInput")
    cb1_t = nc.dram_tensor("cb1", [128, 1], F32, kind="ExternalInput")
    cb2_t = nc.dram_tensor("cb2", [128, 1], F32, kind="ExternalInput")
    ident_t = nc.dram_tensor("ident", [32, 32], BF16, kind="ExternalInput")
    out_t = nc.dram_tensor("out", [4, NOUT], F32, kind="ExternalOutput")

    # ---- internal DRAM (collective bounce buffers) ----
    a2a_in = nc.dram_tensor("a2a_in", [C2, 4, HH * HW], BF16)
    a2a_out = nc.dram_tensor("a2a_out", [N_CORES, 4, 4, HH * HW], BF16)
    rs_in = nc.dram_tensor("rs_in", [B, H1], F32)
    rs_out = nc.dram_tensor("rs_out", [4, H1], F32)

    groups = [list(range(N_CORES))]

    with tile.TileContext(nc) as tc:
        with (
            tc.tile_pool(name="wpool", bufs=w_bufs) as wpool,
            tc.tile_pool(name="const", bufs=1) as cpool,
            tc.tile_pool(name="ps", bufs=4, space="PSUM") as pspool,
            tc.tile_pool(name="work", bufs=2) as wkpool,
            tc.tile_pool(name="persist", bufs=1) as pers,
        ):
            # -------- latency-critical loads first (DMA queue order) ------
            # conv1 input: partition (dy*3+dx)*8 + h holds
            # x_pad[img(h), y0(h)+dy+y, dx+c]; row-quarters double-buffered.
            x9_tiles = []
            for q in range(4):
                x9q = wkpool.tile([72, 28, 232], BF16, tag="x9")
                nc.sync.dma_start(out=x9q[:, :, :],
                                  in_=x9_t[:, 28 * q:28 * q + 28, :])
                x9_tiles.append(x9q)

            s1_sb = cpool.tile([72, 128], BF16, tag="s1")
            nc.sync.dma_start(out=s1_sb[:, :], in_=s1_t[:, :])
            s2_sb = cpool.tile([128, 6, 128], BF16, tag="s2")
            nc.sync.dma_start(out=s2_sb[:, :, :],
                              in_=s2_t[:, :, :].rearrange("t p m -> p t m"))
            b1t_sb = cpool.tile([128, 4, 4], F32, tag="b1t")
            nc.sync.dma_start(out=b1t_sb[:, :, :], in_=b1t_t[:, :, :])
            w2t_sb = cpool.tile([128, 4, 4], F32, tag="w2t")
            nc.sync.dma_start(out=w2t_sb[:, :, :],
                              in_=w2t_t[:, :].rearrange("(k p) o -> p k o", p=128))
            b2t_sb = cpool.tile([4, 4], F32, tag="b2t")
            nc.sync.dma_start(out=b2t_sb[:, :], in_=b2t_t[:, :])
            cb1_sb = cpool.tile([128, 1], F32, tag="cb1")
            nc.sync.dma_start(out=cb1_sb[:, :], in_=cb1_t[:, :])
            cb2_sb = cpool.tile([128, 1], F32, tag="cb2")
            nc.sync.dma_start(out=cb2_sb[:, :], in_=cb2_t[:, :])
            ident_sb = cpool.tile([32, 32], BF16, tag="ident")
            nc.sync.dma_start(out=ident_sb[:, :], in_=ident_t[:, :])

            # conv2 input buffer: partition e*64 + img*16 + ch holds the
            # padded channel image, dx-shifted by e.  Cleared early so the
            # repack DMAs can land as soon as pool1 rows exist.
            c2in = pers.tile([128, 114, 116], BF16, tag="bigC")
            nc.gpsimd.memset(c2in[:, :, :].rearrange("p a b -> p (a b)"), 0.0)

            # -------- fc1 weight stream (fills SBUF buffer from t=0) ------
            wts = []
            for k in range(NK):
                wt = wpool.tile([128, H1], BF16, tag="w")
                nc.scalar.dma_start(out=wt[:, :],
                                    in_=w1t_t[128 * k:128 * k + 128, :])
                wts.append(wt)

            # ---------------- conv1 + pool1 + relu ----------------
            # out partition m = h*16 + oc = img*32 + half*16 + oc
            pool1_a = pers.tile([128, 28, PW], BF16, tag="bigB1")
            pool1_b = pers.tile([128, 28, PW], BF16, tag="bigB2")
            pool1_parts = [pool1_a, pool1_b]
            for T in range(28):            # 2 pooled rows per psum tile
                ps = pspool.tile([128, 2, 512], F32, tag="ps")
                for g in range(2):
                    yp = T * 2 + g         # pooled row within half
                    q, ypl = yp // 14, yp % 14
                    nc.tensor.matmul(
                        ps[:, g, 0:448],
                        lhsT=s1_sb[:, :],
                        rhs=x9_tiles[q][:, 2 * ypl:2 * ypl + 2, :224],
                        start=True, stop=True)
                v = ps[:, :, 0:448].rearrange("p g (r x w) -> p g r x w",
                                              r=2, w=2)
                c1 = wkpool.tile([128, 2, 2, 112], F32, tag="mc")
                nc.scalar.activation(c1[:, :, :, :], v[:, :, :, :, 1], COPY)
                m1 = wkpool.tile([128, 2, 2, 112], F32, tag="mx")
                nc.vector.tensor_max(m1[:, :, :, :], v[:, :, :, :, 0],
                                     c1[:, :, :, :])
                m2 = wkpool.tile([128, 2, 112], F32, tag="mxb")
                nc.vector.tensor_max(m2[:, :, :], m1[:, :, 0, :],
                                     m1[:, :, 1, :])
                half_t, row_t = divmod(2 * T, 28)
                nc.scalar.activation(
                    pool1_parts[half_t][:, row_t:row_t + 2, :],
                    m2[:, :, :], RELU, bias=cb1_sb[:, :])

            if stop_after == "conv1":
                dbg = wkpool.tile([4, NOUT], F32, tag="outsb")
                nc.vector.tensor_copy(dbg[:, :], pool1_a[0:4, 0, 0:4])
                nc.sync.dma_start(out=out_t[:, :], in_=dbg[:, :])
                return

            # -------- repack pool1 -> conv2 input (padded, merged halves,
            # two dx-shifted copies), in two row chunks for overlap --------
            for chunk in range(2):
                r0 = 28 * chunk
                for img in range(4):
                    for half in range(2):
                        srcp = 32 * img + 16 * half
                        for e in range(2):
                            dst = 64 * e + 16 * img
                            nc.sync.dma_start(
                                out=c2in[dst:dst + 16,
                                         56 * half + 1 + r0:
                                         56 * half + 29 + r0,
                                         1 - e:113 - e],
                                in_=pool1_parts[chunk][srcp:srcp + 16, :, :])

            # ---------------- conv2 + pool2 + relu ----------------
            # out partition m = img*32 + oc; 6 passes t=(dy, grp):
            # partition block e supplies tap dx = 2*grp + e.
            h_sb = pers.tile([128, 7, 4, 2, 56], BF16, tag="bigD")
            for T in range(14):            # 8 conv rows / 4 pooled rows
                ps = pspool.tile([128, 2, 512], F32, tag="ps")
                for sub in range(2):
                    y0 = 8 * T + 4 * sub
                    for t in range(6):
                        dy, grp = t // 2, t % 2
                        nc.tensor.matmul(
                            ps[:, sub, 0:448],
                            lhsT=s2_sb[:, t, :],
                            rhs=c2in[:, y0 + dy:y0 + dy + 4,
                                     2 * grp:2 * grp + 112],
                            start=(t == 0), stop=(t == 5))
                v = ps[:, :, 0:448].rearrange("p s (r x w) -> p s r x w",
                                              r=4, w=2)
                c1 = wkpool.tile([128, 2, 4, 56], F32, tag="mc")
                nc.scalar.activation(c1[:, :, :, :], v[:, :, :, :, 1], COPY)
                m1 = wkpool.tile([128, 2, 4, 56], F32, tag="mx")
                nc.vector.tensor_max(m1[:, :, :, :], v[:, :, :, :, 0],
                                     c1[:, :, :, :])
                v2 = m1[:, :, :, :].rearrange("p s (rp w) x -> p s rp w x",
                                              w=2)
                m2 = wkpool.tile([128, 2, 2, 56], F32, tag="mxb")
                nc.vector.tensor_max(m2[:, :, :, :], v2[:, :, :, 0, :],
                                     v2[:, :, :, 1, :])
                # pooled rows 4T..4T+4 -> h_sb[T//2, 2*(T%2) + (0..1), ...]
                nc.scalar.activation(
                    h_sb[:, T // 2, 2 * (T % 2):2 * (T % 2) + 2, :, :],
                    m2[:, :, :, :], RELU, bias=cb2_sb[:, :])

            if stop_after == "conv2":
                dbg = wkpool.tile([4, NOUT], F32, tag="outsb")
                nc.vector.tensor_copy(dbg[:, :], h_sb[0:4, 0, 0, 0, 0:4])
                nc.sync.dma_start(out=out_t[:, :], in_=dbg[:, :])
                return

            # -------- AllToAll: batch-shard -> feature-shard --------
            for img in range(4):
                nc.sync.dma_start(
                    out=a2a_in[:, img, :],
                    in_=h_sb[32 * img:32 * img + 32, :, :, :, :].rearrange(
                        "p a b c d -> p (a b c d)"))
            nc.gpsimd.collective_compute(
                "AllToAll", mybir.AluOpType.bypass, replica_groups=groups,
                ins=[a2a_in[:, :, :]], outs=[a2a_out[:, :, :, :]])

            # h_all partition img (0..31) holds all 12544 local features
            h_all = pers.tile([32, FSH], BF16, tag="bigBall")
            for i in range(N_CORES):
                nc.sync.dma_start(
                    out=h_all[4 * i:4 * i + 4, :].rearrange(
                        "p (o s) -> p o s", o=4),
                    in_=a2a_out[i, :, :, :].rearrange("o i s -> i o s"))

            if stop_after == "a2a":
                dbg = wkpool.tile([4, NOUT], F32, tag="outsb")
                nc.vector.tensor_copy(dbg[:, :], h_all[0:4, 0:4])
                nc.sync.dma_start(out=out_t[:, :], in_=dbg[:, :])
                return

            # -------- transpose h_all -> hT (feature-major) --------
            hT = pers.tile([128, NK, 32], BF16, tag="bigD")
            tp_a = pspool.tile([128, 64, 32], BF16, tag="ps")
            tp_b = pspool.tile([128, 64, 32], BF16, tag="ps")
            tp_tiles = [tp_a, tp_b]
            for k in range(NK):
                tp = tp_tiles[k // 64]
                nc.tensor.transpose(
                    tp[:, k % 64, :],
                    h_all[0:32, 128 * k:128 * k + 128],
                    ident_sb[0:32, :])
            nc.vector.tensor_copy(hT[:, 0:64, :], tp_tiles[0][:, :, :])
            nc.vector.tensor_copy(hT[:, 64:NK, :],
                                  tp_tiles[1][:, 0:NK - 64, :])

            if stop_after == "transpose":
                dbg = wkpool.tile([4, NOUT], F32, tag="outsb")
                nc.vector.tensor_copy(dbg[:, :], hT[0:4, 0, 0:4])
                nc.sync.dma_start(out=out_t[:, :], in_=dbg[:, :])
                return

            # ---------------- fc1 partial ----------------
            fc1_ps = pspool.tile([32, H1], F32, tag="ps")
            for k in range(NK):
                nc.tensor.matmul(fc1_ps[:, :], lhsT=hT[:, k, :],
                                 rhs=wts[k][:, :],
                                 start=(k == 0), stop=(k == NK - 1))
            fc1_sb = wkpool.tile([B, H1], F32, tag="fc1")
            nc.vector.tensor_copy(fc1_sb[:, :], fc1_ps[:, :])
            nc.sync.dma_start(out=rs_in[:, :], in_=fc1_sb[:, :])

            if stop_after == "fc1":
                nc.sync.dma_start(out=out_t[:, :], in_=fc1_sb[0:4, 0:4])
                return

            # -------- ReduceScatter + bias + relu + fc2 --------
            nc.gpsimd.collective_compute(
                "ReduceScatter", mybir.AluOpType.add, replica_groups=groups,
                ins=[rs_in[:, :]], outs=[rs_out[:, :]])

            h2t = wkpool.tile([128, 4, 4], F32, tag="h2t")   # [c, k, img]
            for k in range(4):
                nc.sync.dma_start(
                    out=h2t[:, k, :],
                    in_=rs_out[:, 128 * k:128 * k + 128].rearrange(
                        "i p -> p i"))
            nc.vector.tensor_add(h2t[:, :, :], h2t[:, :, :], b1t_sb[:, :, :])
            nc.scalar.activation(h2t[:, :, :], h2t[:, :, :], RELU)

            fc2_ps = pspool.tile([4, 4], F32, tag="ps")
            for k in range(4):
                nc.tensor.matmul(fc2_ps[:, :], lhsT=h2t[:, k, :],
                                 rhs=w2t_sb[:, k, :],
                                 start=(k == 0), stop=(k == 3))
            out_sb = wkpool.tile([4, NOUT], F32, tag="outsb")
            nc.vector.tensor_add(out_sb[:, :], fc2_ps[:, :], b2t_sb[:, :])
            nc.sync.dma_start(out=out_t[:, :], in_=out_sb[:, :])


def _get_program(w_bufs: int = 94):
    key = ("prog", w_bufs)
    if key not in _CACHE:
        _CACHE[key] = _build_program(w_bufs)
    return _CACHE[key]


def _host_prep(x, conv1_w, conv1_b, conv2_w, conv2_b, values, w_idx1,
               fc1_b, w_idx2, fc2_b):
    """Build per-core input maps (numpy, bf16 for PE-facing tensors)."""
    f32 = np.float32
    x = np.asarray(x, f32)
    conv1_w = np.asarray(conv1_w, f32)
    conv2_w = np.asarray(conv2_w, f32)
    values = np.asarray(values, f32)
    w_idx1 = np.asarray(w_idx1)
    w_idx2 = np.asarray(w_idx2)

    x_pad = np.zeros((B, 226, 232), f32)
    x_pad[:, 1:225, 1:225] = x[:, 0]

    # x9[c]: [72, 112, 232]; partition (dy*3+dx)*8 + h, h = 2*img_loc + half
    x9 = np.zeros((N_CORES, 72, PH, 232), f32)
    for dy in range(3):
        for dx in range(3):
            for h in range(8):
                il, half = h // 2, h % 2
                y0 = PH * half
                for c in range(N_CORES):
                    x9[c, (dy * 3 + dx) * 8 + h, :, :232 - dx] = \
                        x_pad[4 * c + il, y0 + dy:y0 + dy + PH, dx:]

    s1 = np.zeros((72, 128), f32)
    for dy in range(3):
        for dx in range(3):
            for h in range(8):
                s1[(dy * 3 + dx) * 8 + h, 16 * h:16 * h + C1] = \
                    conv1_w[:, 0, dy, dx]

    # conv2 stationaries [6, 128, 128]: pass t = dy*2 + grp;
    # partition p = e*64 + img*16 + ch supplies tap dx = 2*grp + e
    s2 = np.zeros((6, 128, 128), f32)
    for t in range(6):
        dy, grp = t // 2, t % 2
        for e in range(2):
            dx = 2 * grp + e
            if dx > 2:
                continue
            for img in range(4):
                for ch in range(C1):
                    s2[t, 64 * e + 16 * img + ch, 32 * img:32 * img + C2] = \
                        conv2_w[:, ch, dy, dx]

    w1ts = []
    for c in range(N_CORES):
        idx = w_idx1[:, FSH * c:FSH * (c + 1)]             # [512, 12544]
        w1ts.append(np.ascontiguousarray(values[idx].T).astype(BF16NP))

    b1t = np.repeat(np.asarray(fc1_b, f32).reshape(4, 128).T[:, :, None],
                    4, axis=2).copy()                       # [128, k4, img4]
    w2t = np.ascontiguousarray(values[w_idx2].T).astype(f32)  # [512, 4]
    b2t = np.broadcast_to(np.asarray(fc2_b, f32), (4, 4)).copy()

    cb1 = np.zeros((128, 1), f32)
    for h in range(8):
        cb1[16 * h:16 * h + C1, 0] = np.asarray(conv1_b, f32)
    cb2 = np.zeros((128, 1), f32)
    for img in range(4):
        cb2[32 * img:32 * img + C2, 0] = np.asarray(conv2_b, f32)

    ident = np.eye(32, dtype=f32).astype(BF16NP)

    s1 = s1.astype(BF16NP)
    s2 = s2.astype(BF16NP)
    in_maps = []
    for c in range(N_CORES):
        in_maps.append({
            "x9": np.ascontiguousarray(x9[c]).astype(BF16NP),
            "s1": s1, "s2": s2,
            "w1t": w1ts[c],
            "b1t": b1t, "w2t": w2t, "b2t": b2t,
            "cb1": cb1, "cb2": cb2, "ident": ident,
        })
    return in_maps


def kernel(x, conv1_w, conv1_b, conv2_w, conv2_b, values, w_idx1, fc1_b,
           w_idx2, fc2_b, _trace=False, _trace_kwargs=None):
    nc = _get_program()
    in_maps = _host_prep(x, conv1_w, conv1_b, conv2_w, conv2_b, values,
                         w_idx1, fc1_b, w_idx2, fc2_b)
    res = run_bass_kernel_spmd(nc, in_maps, core_ids=list(range(N_CORES)),
                               trace=_trace, **(_trace_kwargs or {}))
    out = np.zeros((B, NOUT), np.float32)
    for c in range(N_CORES):
        out[4 * c:4 * c + 4] = res.results[c]["out"]
    if _trace:
        kernel.last_result = res
    return out


if __name__ == "__main__":
    rng = np.random.default_rng(0)
    ins = {
        "x": rng.standard_normal((B, 1, IMG, IMG), dtype=np.float32),
        "conv1_w": rng.standard_normal((16, 1, 3, 3), dtype=np.float32) * 0.1,
        "conv1_b": np.zeros(16, np.float32),
        "conv2_w": rng.standard_normal((32, 16, 3, 3), dtype=np.float32) * 0.05,
        "conv2_b": np.zeros(32, np.float32),
        "values": np.sort(rng.standard_normal(4096).astype(np.float32) * 0.01),
        "w_idx1": rng.integers(0, 4096, (512, FEAT), dtype=np.int32),
        "fc1_b": np.zeros(512, np.float32),
        "w_idx2": rng.integers(0, 4096, (4, 512), dtype=np.int32),
        "fc2_b": np.zeros(4, np.float32),
    }
    out = kernel(**ins)
    print("out shape", out.shape, "sample row", out[0])



# revision 7
# speedup vs baseline: 1.3017x; 1.3017x over previous
"""Trainium2 Bass kernel for nn_MemristorCNN (embedding_lookup, 8 cores).

Strategy:
- Host gathers W1 = values[w_idx1], ships bf16 column-shard [12544, 512]
  per core; streamed as 7 big DMAs on the scalar HWDGE ring from t=0.
- Conv stack data-parallel (4 images/core); conv1 packs (tap, img-half)
  into K=72 with one pass; conv2 packs (dx-parity, img, ic) into K=128
  with 6 tap passes.  Pool chains are 1 ACT (relu+bias+bf16) + 2 DVE
  maxes per tile, engine-parallel.
- AllToAll redistributes h to feature shards (channels 4c..4c+4); conv2
  output partition order (oc, img) makes staging one DMA per dest.
- PE transposes h to feature-major; fc1 partial matmul; ReduceScatter;
  relu + fc2 on device; host concatenates per-core [4, 4] outputs.
"""

import sys

import numpy as np
import ml_dtypes

BF16NP = ml_dtypes.bfloat16

for _p in ("/opt/trn_rl_repo",):
    if _p not in sys.path:
        sys.path.insert(0, _p)

import concourse.bacc as bacc
import concourse.bass as bass  # noqa: F401
import concourse.tile as tile
from concourse import mybir
from concourse.bass_utils import run_bass_kernel_spmd

F32 = mybir.dt.float32
BF16 = mybir.dt.bfloat16
RELU = mybir.ActivationFunctionType.Relu

N_CORES = 8
B = 32
IMG = 224
C1, C2 = 16, 32
PH = 112
HH = 56
FEAT = C2 * HH * HH          # 100352
FSH = FEAT // N_CORES        # 12544
NK = FSH // 128              # 98
WCH = 7                      # weight stream chunks
KPC = NK // WCH              # 14 k-tiles per chunk
H1 = 512
NOUT = 4

_CACHE = {}


def _build_program(stop_after: str = 'full'):
    nc = bacc.Bacc("TRN2", target_bir_lowering=False, debug=False,
                   num_devices=N_CORES)
    _emit(nc, stop_after)
    nc.compile()
    return nc


def _emit(nc, stop_after: str):
    # ---- kernel I/O ----
    x9_t = nc.dram_tensor("x9", [72, PH, 232], BF16, kind="ExternalInput")
    s1_t = nc.dram_tensor("s1", [72, 128], BF16, kind="ExternalInput")
    s2_t = nc.dram_tensor("s2", [6, 128, 128], BF16, kind="ExternalInput")
    w1t_t = nc.dram_tensor("w1t", [FSH, H1], BF16, kind="ExternalInput")
    b1t_t = nc.dram_tensor("b1t", [128, 4, 4], F32, kind="ExternalInput")
    w2t_t = nc.dram_tensor("w2t", [H1, NOUT], F32, kind="ExternalInput")
    b2t_t = nc.dram_tensor("b2t", [4, 4], F32, kind="ExternalInput")
    cb1_t = nc.dram_tensor("cb1", [128, 1], F32, kind="ExternalInput")
    cb2_t = nc.dram_tensor("cb2", [128, 1], F32, kind="ExternalInput")
    ident_t = nc.dram_tensor("ident", [32, 32], BF16, kind="ExternalInput")
    out_t = nc.dram_tensor("out", [4, NOUT], F32, kind="ExternalOutput")

    # ---- internal DRAM (collective bounce buffers) ----
    a2a_in = nc.dram_tensor("a2a_in", [N_CORES, 16, HH * HH], BF16)
    a2a_out = nc.dram_tensor("a2a_out", [N_CORES, 16, HH * HH], BF16)
    rs_in = nc.dram_tensor("rs_in", [B, H1], F32)
    rs_out = nc.dram_tensor("rs_out", [4, H1], F32)

    groups = [list(range(N_CORES))]

    with tile.TileContext(nc) as tc:
        with (
            tc.tile_pool(name="wpool", bufs=WCH) as wpool,
            tc.tile_pool(name="const", bufs=1) as cpool,
            tc.tile_pool(name="ps", bufs=4, space="PSUM") as pspool,
            tc.tile_pool(name="xin", bufs=2) as xpool,
            tc.tile_pool(name="act", bufs=2) as apool,
            tc.tile_pool(name="mx", bufs=2) as mpool,
            tc.tile_pool(name="big", bufs=1) as bigpool,
            tc.tile_pool(name="mid", bufs=1) as midpool,
            tc.tile_pool(name="pers", bufs=1) as pers,
            tc.tile_pool(name="sm", bufs=2) as smpool,
        ):
            # -------- input loads (sync HWDGE ring) --------
            x9_tiles = [None] * 4
            for q in range(2):
                x9q = xpool.tile([72, 28, 232], BF16, tag="x9")
                nc.sync.dma_start(out=x9q[:, :, :],
                                  in_=x9_t[:, 28 * q:28 * q + 28, :])
                x9_tiles[q] = x9q

            s1_sb = cpool.tile([72, 128], BF16, tag="s1")
            nc.sync.dma_start(out=s1_sb[:, :], in_=s1_t[:, :])
            s2_sb = cpool.tile([128, 6, 128], BF16, tag="s2")
            nc.sync.dma_start(out=s2_sb[:, :, :],
                              in_=s2_t[:, :, :].rearrange("t p m -> p t m"))
            cb1_sb = cpool.tile([128, 1], F32, tag="cb1")
            nc.sync.dma_start(out=cb1_sb[:, :], in_=cb1_t[:, :])
            cb2_sb = cpool.tile([128, 1], F32, tag="cb2")
            nc.sync.dma_start(out=cb2_sb[:, :], in_=cb2_t[:, :])
            ident_sb = cpool.tile([32, 32], BF16, tag="ident")
            nc.sync.dma_start(out=ident_sb[:, :], in_=ident_t[:, :])
            b1t_sb = cpool.tile([128, 4, 4], F32, tag="b1t")
            nc.sync.dma_start(out=b1t_sb[:, :, :], in_=b1t_t[:, :, :])
            w2t_sb = cpool.tile([128, 4, 4], F32, tag="w2t")
            nc.sync.dma_start(out=w2t_sb[:, :, :],
                              in_=w2t_t[:, :].rearrange("(k p) o -> p k o", p=128))
            b2t_sb = cpool.tile([4, 4], F32, tag="b2t")
            nc.sync.dma_start(out=b2t_sb[:, :], in_=b2t_t[:, :])

            for q in range(2, 4):
                x9q = xpool.tile([72, 28, 232], BF16, tag="x9")
                nc.sync.dma_start(out=x9q[:, :, :],
                                  in_=x9_t[:, 28 * q:28 * q + 28, :])
                x9_tiles[q] = x9q

            # -------- fc1 weight stream (scalar HWDGE ring, from t=0) ----
            wts = []
            for cch in range(WCH):
                wt = wpool.tile([128, KPC, H1], BF16, tag="w")
                nc.scalar.dma_start(
                    out=wt[:, :, :],
                    in_=w1t_t[128 * KPC * cch:128 * KPC * (cch + 1), :]
                    .rearrange("(k p) h -> p k h", p=128))
                wts.append(wt)

            # conv2 input: partition e*64 + img*16 + ic; padded borders.
            c2in = bigpool.tile([128, 114, 116], BF16, tag="big")
            nc.gpsimd.memset(c2in[:, 0, :], 0.0)
            nc.gpsimd.memset(c2in[:, 113, :], 0.0)
            nc.gpsimd.memset(c2in[:, 1:113, 0:1], 0.0)
            nc.gpsimd.memset(c2in[:, 1:113, 112:114], 0.0)

            # conv1 output: partition half*64 + img*16 + oc, rows local to
            # the half (56 pooled rows), 112 pooled cols.
            pool1 = midpool.tile([128, HH, PH], BF16, tag="mid")
            # conv2 output: partition oc*4 + img, [56, 56] pooled.
            h_sb = pers.tile([128, HH, HH], BF16, tag="hsb")

            def conv1_step(T):
                ps = pspool.tile([128, 2, 512], F32, tag="ps")
                for g in range(2):
                    yp = 2 * T + g
                    q, ypl = yp // 14, yp % 14
                    nc.tensor.matmul(
                        ps[:, g, 0:448],
                        lhsT=s1_sb[:, :],
                        rhs=x9_tiles[q][:, 2 * ypl:2 * ypl + 2, :224],
                        start=True, stop=True)
                v = ps[:, :, 0:448].rearrange("p g (r x w) -> p g r x w",
                                          r=2, w=2)
                t1 = apool.tile([128, 2, 2, 112, 2], BF16, tag="act")
                nc.scalar.activation(t1[:, :, :, :, :], v[:, :, :, :, :],
                                     RELU, bias=cb1_sb[:, :])
                m1 = mpool.tile([128, 2, 112, 2], BF16, tag="mx")
                nc.vector.tensor_max(m1[:, :, :, :], t1[:, :, 0, :, :],
                                     t1[:, :, 1, :, :])
                nc.vector.tensor_max(pool1[:, 2 * T:2 * T + 2, :],
                                     m1[:, :, :, 0], m1[:, :, :, 1])

            def repack_chunk(ck):
                r0 = 14 * ck
                for h in range(2):
                    for e in range(2):
                        nc.sync.dma_start(
                            out=c2in[64 * e:64 * e + 64,
                                     56 * h + 1 + r0:56 * h + 15 + r0,
                                     1 - e:113 - e],
                            in_=pool1[64 * h:64 * h + 64, r0:r0 + 14, :])

            def conv2_step(j):
                ps = pspool.tile([128, 2, 512], F32, tag="ps")
                for sub in range(2):
                    y0 = 8 * j + 4 * sub
                    for t in range(6):
                        dy, grp = t // 2, t % 2
                        nc.tensor.matmul(
                            ps[:, sub, 0:448],
                            lhsT=s2_sb[:, t, :],
                            rhs=c2in[:, y0 + dy:y0 + dy + 4,
                                     2 * grp:2 * grp + 112],
                            start=(t == 0), stop=(t == 5))
                v = ps[:, :, 0:448].rearrange("p s (r x w) -> p s r x w",
                                          r=4, w=2)
                t2 = apool.tile([128, 2, 4, 56, 2], BF16, tag="act")
                nc.scalar.activation(t2[:, :, :, :, :], v[:, :, :, :, :],
                                     RELU, bias=cb2_sb[:, :])
                t2v = t2[:, :, :, :, :].rearrange("p s (q rr) x w -> p s q rr x w",
                                                  q=2)
                m2 = mpool.tile([128, 2, 2, 56, 2], BF16, tag="mx")
                nc.vector.tensor_max(m2[:, :, :, :, :], t2v[:, :, :, 0, :, :],
                                     t2v[:, :, :, 1, :, :])
                hv = h_sb[:, 4 * j:4 * j + 4, :].rearrange(
                    "p (s q) x -> p s q x", s=2)
                nc.vector.tensor_max(hv[:, :, :, :], m2[:, :, :, :, 0],
                                     m2[:, :, :, :, 1])

            if stop_after in ("ps0", "c1t0"):
                ps = pspool.tile([128, 2, 512], F32, tag="ps")
                for g in range(2):
                    yp = g
                    nc.tensor.matmul(
                        ps[:, g, 0:448], lhsT=s1_sb[:, :],
                        rhs=x9_tiles[0][:, 2 * yp:2 * yp + 2, :224],
                        start=True, stop=True)
                dbg = smpool.tile([4, NOUT], F32, tag="outsb")
                if stop_after == "ps0":
                    nc.vector.tensor_copy(dbg[:, :], ps[0:4, 0, 0:4])
                else:
                    v = ps[:, :, 0:448].rearrange("p g (r x w) -> p g r x w",
                                              r=2, w=2)
                    t1 = apool.tile([128, 2, 2, 112, 2], BF16, tag="act")
                    nc.scalar.activation(t1[:, :, :, :, :], v[:, :, :, :, :],
                                         RELU, bias=cb1_sb[:, :])
                    m1 = mpool.tile([128, 2, 112, 2], BF16, tag="mx")
                    nc.vector.tensor_max(m1[:, :, :, :], t1[:, :, 0, :, :],
                                         t1[:, :, 1, :, :])
                    nc.vector.tensor_max(pool1[:, 0:2, :],
                                         m1[:, :, :, 0], m1[:, :, :, 1])
                    nc.vector.tensor_copy(dbg[:, :], pool1[0:4, 0, 0:4])
                nc.sync.dma_start(out=out_t[:, :], in_=dbg[:, :])
                return

            # interleave conv1 / repack / conv2 for engine overlap
            for T in range(7):
                conv1_step(T)
            repack_chunk(0)
            for T in range(7, 14):
                conv1_step(T)
            repack_chunk(1)
            for j in range(3):
                conv2_step(j)
            for T in range(14, 21):
                conv1_step(T)
            repack_chunk(2)
            for j in range(3, 5):
                conv2_step(j)
            for T in range(21, 28):
                conv1_step(T)
            repack_chunk(3)
            for j in range(5, 14):
                conv2_step(j)

            if stop_after == "conv1":
                dbg = smpool.tile([4, NOUT], F32, tag="outsb")
                nc.vector.tensor_copy(dbg[:, :], pool1[0:4, 0, 0:4])
                nc.sync.dma_start(out=out_t[:, :], in_=dbg[:, :])
                return

            if stop_after == "conv2":
                dbg = smpool.tile([4, NOUT], F32, tag="outsb")
                nc.vector.tensor_copy(dbg[:, :], h_sb[0:4, 0, 0:4])
                nc.sync.dma_start(out=out_t[:, :], in_=dbg[:, :])
                return

            # -------- AllToAll: batch-shard -> feature-shard --------
            for d in range(N_CORES):
                nc.sync.dma_start(
                    out=a2a_in[d, :, :],
                    in_=h_sb[16 * d:16 * d + 16, :, :].rearrange(
                        "p a b -> p (a b)"))
            nc.gpsimd.collective_compute(
                "AllToAll", mybir.AluOpType.bypass, replica_groups=groups,
                ins=[a2a_in[:, :, :]], outs=[a2a_out[:, :, :]])

            # h_all partition = global image (0..31), all 12544 local feats
            h_all = bigpool.tile([32, FSH], BF16, tag="big")
            for s in range(N_CORES):
                nc.sync.dma_start(
                    out=h_all[4 * s:4 * s + 4, :].rearrange(
                        "p (j sp) -> p j sp", j=4),
                    in_=a2a_out[s, :, :].rearrange("(j i) sp -> i j sp",
                                                   j=4))

            if stop_after == "a2a":
                dbg = smpool.tile([4, NOUT], F32, tag="outsb")
                nc.vector.tensor_copy(dbg[:, :], h_all[0:4, 0:4])
                nc.sync.dma_start(out=out_t[:, :], in_=dbg[:, :])
                return

            # -------- transpose h_all -> hT (feature-major) --------
            hT = midpool.tile([128, NK, 32], BF16, tag="mid")
            for half in range(2):
                tp = pspool.tile([128, 49, 32], BF16, tag="ps")
                for kk in range(49):
                    k = 49 * half + kk
                    nc.tensor.transpose(
                        tp[:, kk, :],
                        h_all[0:32, 128 * k:128 * k + 128],
                        ident_sb[0:32, :])
                nc.vector.tensor_copy(hT[:, 49 * half:49 * half + 49, :],
                                      tp[:, :, :])

            # ---------------- fc1 partial ----------------
            fc1_ps = pspool.tile([32, H1], F32, tag="ps")
            for k in range(NK):
                nc.tensor.matmul(fc1_ps[:, :], lhsT=hT[:, k, :],
                                 rhs=wts[k // KPC][:, k % KPC, :],
                                 start=(k == 0), stop=(k == NK - 1))
            fc1_sb = smpool.tile([B, H1], F32, tag="fc1")
            nc.vector.tensor_copy(fc1_sb[:, :], fc1_ps[:, :])
            nc.sync.dma_start(out=rs_in[:, :], in_=fc1_sb[:, :])

            if stop_after == "fc1":
                nc.sync.dma_start(out=out_t[:, :], in_=fc1_sb[0:4, 0:4])
                return

            # -------- ReduceScatter + bias + relu + fc2 --------
            nc.gpsimd.collective_compute(
                "ReduceScatter", mybir.AluOpType.add, replica_groups=groups,
                ins=[rs_in[:, :]], outs=[rs_out[:, :]])

            h2t = smpool.tile([128, 4, 4], F32, tag="h2t")   # [p, k, img]
            for k in range(4):
                nc.sync.dma_start(
                    out=h2t[:, k, :],
                    in_=rs_out[:, 128 * k:128 * k + 128].rearrange(
                        "i p -> p i"))
            nc.vector.tensor_add(h2t[:, :, :], h2t[:, :, :], b1t_sb[:, :, :])
            nc.scalar.activation(h2t[:, :, :], h2t[:, :, :], RELU)

            fc2_ps = pspool.tile([4, 4], F32, tag="ps")
            for k in range(4):
                nc.tensor.matmul(fc2_ps[:, :], lhsT=h2t[:, k, :],
                                 rhs=w2t_sb[:, k, :],
                                 start=(k == 0), stop=(k == 3))
            out_sb = smpool.tile([4, NOUT], F32, tag="outsb")
            nc.vector.tensor_add(out_sb[:, :], fc2_ps[:, :], b2t_sb[:, :])
            nc.sync.dma_start(out=out_t[:, :], in_=out_sb[:, :])


def _get_program(stop_after: str = 'full'):
    key = ("prog", stop_after)
    if key not in _CACHE:
        _CACHE[key] = _build_program(stop_after)
    return _CACHE[key]


def _host_prep(x, conv1_w, conv1_b, conv2_w, conv2_b, values, w_idx1,
               fc1_b, w_idx2, fc2_b):
    """Build per-core input maps (numpy, bf16 for PE-facing tensors)."""
    f32 = np.float32
    x = np.asarray(x, f32)
    conv1_w = np.asarray(conv1_w, f32)
    conv2_w = np.asarray(conv2_w, f32)
    values = np.asarray(values, f32)
    w_idx1 = np.asarray(w_idx1)
    w_idx2 = np.asarray(w_idx2)

    x_pad = np.zeros((B, 226, 232), f32)
    x_pad[:, 1:225, 1:225] = x[:, 0]

    # x9[c]: [72, 112, 232]; partition (dy*3+dx)*8 + h, h = 2*img_loc + half
    x9 = np.zeros((N_CORES, 72, PH, 232), f32)
    for dy in range(3):
        for dx in range(3):
            for h in range(8):
                il, half = h // 2, h % 2
                y0 = PH * half
                for c in range(N_CORES):
                    x9[c, (dy * 3 + dx) * 8 + h, :, :232 - dx] = \
                        x_pad[4 * c + il, y0 + dy:y0 + dy + PH, dx:]

    # conv1 stationary: M-order = half*64 + img*16 + oc
    s1 = np.zeros((72, 128), f32)
    for dy in range(3):
        for dx in range(3):
            for h in range(8):
                il, half = h // 2, h % 2
                m0 = 64 * half + 16 * il
                s1[(dy * 3 + dx) * 8 + h, m0:m0 + C1] = conv1_w[:, 0, dy, dx]

    # conv2 stationaries [6, 128, 128]: pass t = dy*2 + grp;
    # row p = e*64 + img*16 + ic supplies tap dx = 2*grp + e;
    # col q = oc*4 + img (block-diagonal in img)
    s2 = np.zeros((6, 128, 128), f32)
    for t in range(6):
        dy, grp = t // 2, t % 2
        for e in range(2):
            dx = 2 * grp + e
            if dx > 2:
                continue
            for img in range(4):
                for ic in range(C1):
                    s2[t, 64 * e + 16 * img + ic,
                       img::4][:C2] = conv2_w[:, ic, dy, dx]

    w1ts = []
    for c in range(N_CORES):
        idx = w_idx1[:, FSH * c:FSH * (c + 1)]             # [512, 12544]
        w1ts.append(np.ascontiguousarray(values[idx].T).astype(BF16NP))

    b1t = np.repeat(np.asarray(fc1_b, f32).reshape(4, 128).T[:, :, None],
                    4, axis=2).copy()                       # [128, k4, img4]
    w2t = np.ascontiguousarray(values[w_idx2].T).astype(f32)  # [512, 4]
    b2t = np.broadcast_to(np.asarray(fc2_b, f32), (4, 4)).copy()

    cb1 = np.zeros((128, 1), f32)
    for half in range(2):
        for il in range(4):
            m0 = 64 * half + 16 * il
            cb1[m0:m0 + C1, 0] = np.asarray(conv1_b, f32)
    cb2 = np.zeros((128, 1), f32)
    for oc in range(C2):
        cb2[4 * oc:4 * oc + 4, 0] = conv2_b[oc]

    ident = np.eye(32, dtype=f32).astype(BF16NP)

    s1 = s1.astype(BF16NP)
    s2 = s2.astype(BF16NP)
    in_maps = []
    for c in range(N_CORES):
        in_maps.append({
            "x9": np.ascontiguousarray(x9[c]).astype(BF16NP),
            "s1": s1, "s2": s2,
            "w1t": w1ts[c],
            "b1t": b1t, "w2t": w2t, "b2t": b2t,
            "cb1": cb1, "cb2": cb2, "ident": ident,
        })
    return in_maps


def kernel(x, conv1_w, conv1_b, conv2_w, conv2_b, values, w_idx1, fc1_b,
           w_idx2, fc2_b, _trace=False, _trace_kwargs=None,
           _stop_after='full'):
    nc = _get_program(_stop_after)
    in_maps = _host_prep(x, conv1_w, conv1_b, conv2_w, conv2_b, values,
                         w_idx1, fc1_b, w_idx2, fc2_b)
    res = run_bass_kernel_spmd(nc, in_maps, core_ids=list(range(N_CORES)),
                               trace=_trace, **(_trace_kwargs or {}))
    out = np.zeros((B, NOUT), np.float32)
    for c in range(N_CORES):
        out[4 * c:4 * c + 4] = res.results[c]["out"]
    if _trace:
        kernel.last_result = res
    return out


if __name__ == "__main__":
    rng = np.random.default_rng(0)
    ins = {
        "x": rng.standard_normal((B, 1, IMG, IMG), dtype=np.float32),
        "conv1_w": rng.standard_normal((16, 1, 3, 3), dtype=np.float32) * 0.1,
        "conv1_b": np.zeros(16, np.float32),
        "conv2_w": rng.standard_normal((32, 16, 3, 3), dtype=np.float32) * 0.05,
        "conv2_b": np.zeros(32, np.float32),
        "values": np.sort(rng.standard_normal(4096).astype(np.float32) * 0.01),
        "w_idx1": rng.integers(0, 4096, (512, FEAT), dtype=np.int32),
        "fc1_b": np.zeros(512, np.float32),
        "w_idx2": rng.integers(0, 4096, (4, 512), dtype=np.int32),
        "fc2_b": np.zeros(4, np.float32),
    }
    out = kernel(**ins)
    print("out shape", out.shape, "sample row", out[0])


# revision 12
# speedup vs baseline: 1.3218x; 1.0154x over previous
"""Trainium2 Bass kernel for nn_MemristorCNN (embedding_lookup, 8 cores).

Strategy:
- Host gathers W1 = values[w_idx1]; ships a bf16 column-shard per core in
  partition-major layout [128, 100, 512] (100 k-tiles: 25 spatial blocks
  x 4 channels, zero-padded), streamed as 5 big DMAs (128 descriptors
  each) on the scalar HWDGE ring from t=0.
- Conv stack data-parallel (4 images/core); conv1 packs (tap, img-half)
  into K=72; conv2 packs (dx-parity, img, ic) into K=128 with 6 tap
  passes.  Pool chains: 1 ACT (relu+bias+bf16) + 2 DVE maxes per tile.
- The h transpose to feature-major happens on the SENDER, interleaved
  with conv2 (25 PE transposes); the AllToAll then delivers
  ready-to-use feature-major tiles, so the post-collective tail is just
  unpack + fc1 + ReduceScatter + fc2.
"""

import sys

import numpy as np
import ml_dtypes

BF16NP = ml_dtypes.bfloat16

for _p in ("/opt/trn_rl_repo",):
    if _p not in sys.path:
        sys.path.insert(0, _p)

import concourse.bacc as bacc
import concourse.bass as bass  # noqa: F401
import concourse.tile as tile
from concourse import mybir
from concourse.bass_utils import run_bass_kernel_spmd

F32 = mybir.dt.float32
BF16 = mybir.dt.bfloat16
RELU = mybir.ActivationFunctionType.Relu

N_CORES = 8
B = 32
IMG = 224
C1, C2 = 16, 32
PH = 112
HH = 56
FEAT = C2 * HH * HH          # 100352
FSH = FEAT // N_CORES        # 12544
SP = HH * HH                 # 3136 spatial per channel
SPP = 3200                   # padded spatial (25 * 128)
NT = 25                      # spatial transpose blocks
NK = 4 * NT                  # 100 k-tiles (4 local channels x 25 blocks)
WCH = 5                      # weight stream chunks
KPC = NK // WCH              # 20 k-tiles per chunk
H1 = 512
NOUT = 4

_CACHE = {}


def _jb(b):
    """conv2 step after which transpose block b's rows are complete."""
    import math
    rows = math.ceil(128 * (b + 1) / HH)
    return min(13, max(0, math.ceil(rows / 4) - 1))


def _build_program(stop_after: str = 'full'):
    nc = bacc.Bacc("TRN2", target_bir_lowering=False, debug=False,
                   num_devices=N_CORES)
    _emit(nc, stop_after)
    nc.compile()
    return nc


def _emit(nc, stop_after: str):
    # ---- kernel I/O ----
    x9_t = nc.dram_tensor("x9", [72, PH, 232], BF16, kind="ExternalInput")
    s1_t = nc.dram_tensor("s1", [72, 128], BF16, kind="ExternalInput")
    s2_t = nc.dram_tensor("s2", [6, 128, 128], BF16, kind="ExternalInput")
    w1t_t = nc.dram_tensor("w1t", [128, NK, H1], BF16, kind="ExternalInput")
    b1t_t = nc.dram_tensor("b1t", [128, 4, 4], F32, kind="ExternalInput")
    w2t_t = nc.dram_tensor("w2t", [H1, NOUT], F32, kind="ExternalInput")
    b2t_t = nc.dram_tensor("b2t", [4, 4], F32, kind="ExternalInput")
    cb1_t = nc.dram_tensor("cb1", [128, 1], F32, kind="ExternalInput")
    cb2_t = nc.dram_tensor("cb2", [128, 1], F32, kind="ExternalInput")
    ident_t = nc.dram_tensor("ident", [128, 128], BF16, kind="ExternalInput")
    ident4_t = nc.dram_tensor("ident4", [4, 4], F32, kind="ExternalInput")
    out_t = nc.dram_tensor("out", [4, NOUT], F32, kind="ExternalOutput")

    # ---- internal DRAM (collective bounce buffers) ----
    a2a_in = nc.dram_tensor("a2a_in", [N_CORES, 128, 400], BF16)
    a2a_out = nc.dram_tensor("a2a_out", [N_CORES, 128, 400], BF16)
    rs_in = nc.dram_tensor("rs_in", [B, H1], F32)
    rs_out = nc.dram_tensor("rs_out", [4, H1], F32)

    groups = [list(range(N_CORES))]

    with tile.TileContext(nc) as tc:
        with (
            tc.tile_pool(name="wpool", bufs=WCH) as wpool,
            tc.tile_pool(name="const", bufs=1) as cpool,
            tc.tile_pool(name="ps", bufs=4, space="PSUM") as pspool,
            tc.tile_pool(name="xin", bufs=2) as xpool,
            tc.tile_pool(name="act", bufs=2) as apool,
            tc.tile_pool(name="mx", bufs=2) as mpool,
            tc.tile_pool(name="big", bufs=1) as bigpool,
            tc.tile_pool(name="mid", bufs=1) as midpool,
            tc.tile_pool(name="pers", bufs=1) as pers,
            tc.tile_pool(name="sm", bufs=2) as smpool,
        ):
            # -------- input loads (sync HWDGE ring) --------
            x9_tiles = [None] * 4
            for q in range(2):
                x9q = xpool.tile([72, 28, 232], BF16, tag="x9")
                nc.sync.dma_start(out=x9q[:, :, :],
                                  in_=x9_t[:, 28 * q:28 * q + 28, :])
                x9_tiles[q] = x9q

            s1_sb = cpool.tile([72, 128], BF16, tag="s1")
            nc.sync.dma_start(out=s1_sb[:, :], in_=s1_t[:, :])
            s2_sb = cpool.tile([128, 6, 128], BF16, tag="s2")
            nc.sync.dma_start(out=s2_sb[:, :, :],
                              in_=s2_t[:, :, :].rearrange("t p m -> p t m"))
            cb1_sb = cpool.tile([128, 1], F32, tag="cb1")
            nc.sync.dma_start(out=cb1_sb[:, :], in_=cb1_t[:, :])
            cb2_sb = cpool.tile([128, 1], F32, tag="cb2")
            nc.sync.dma_start(out=cb2_sb[:, :], in_=cb2_t[:, :])
            ident_sb = cpool.tile([128, 128], BF16, tag="ident")
            nc.sync.dma_start(out=ident_sb[:, :], in_=ident_t[:, :])
            b1t_sb = cpool.tile([128, 4, 4], F32, tag="b1t")
            nc.sync.dma_start(out=b1t_sb[:, :, :], in_=b1t_t[:, :, :])
            w2t_sb = cpool.tile([128, 4, 4], F32, tag="w2t")
            nc.sync.dma_start(out=w2t_sb[:, :, :],
                              in_=w2t_t[:, :].rearrange("(k p) o -> p k o", p=128))
            b2t_sb = cpool.tile([4, 4], F32, tag="b2t")
            nc.sync.dma_start(out=b2t_sb[:, :], in_=b2t_t[:, :])
            ident4_sb = cpool.tile([4, 4], F32, tag="ident4")
            nc.sync.dma_start(out=ident4_sb[:, :], in_=ident4_t[:, :])

            for q in range(2, 4):
                x9q = xpool.tile([72, 28, 232], BF16, tag="x9")
                nc.sync.dma_start(out=x9q[:, :, :],
                                  in_=x9_t[:, 28 * q:28 * q + 28, :])
                x9_tiles[q] = x9q

            # -------- fc1 weight stream (scalar HWDGE ring, from t=0) ----
            wts = []
            for cch in range(WCH):
                wt = wpool.tile([128, KPC, H1], BF16, tag="w")
                nc.scalar.dma_start(
                    out=wt[:, :, :],
                    in_=w1t_t[:, KPC * cch:KPC * (cch + 1), :])
                wts.append(wt)

            # conv2 input: partition e*64 + img*16 + ic; padded borders.
            c2in = bigpool.tile([128, 114, 116], BF16, tag="big")
            nc.gpsimd.memset(c2in[:, 0, :], 0.0)
            nc.gpsimd.memset(c2in[:, 113, :], 0.0)
            nc.gpsimd.memset(c2in[:, 1:113, 0:1], 0.0)
            nc.gpsimd.memset(c2in[:, 1:113, 112:114], 0.0)

            # conv1 output: partition half*64 + img*16 + oc, 56 rows/half.
            pool1 = midpool.tile([128, HH, PH], BF16, tag="mid")
            # conv2 output: partition img*32 + oc, flat padded spatial.
            h_sb = pers.tile([128, SPP], BF16, tag="hsb")
            nc.gpsimd.memset(h_sb[:, SP:SPP], 0.0)
            # sender-side transposed h: [p_sp, (dest8, img4, blk25, ch4)]
            hTo = pers.tile([128, SPP], BF16, tag="hTo")

            def conv1_step(T):
                ps = pspool.tile([128, 2, 512], F32, tag="ps")
                for g in range(2):
                    yp = 2 * T + g
                    q, ypl = yp // 14, yp % 14
                    nc.tensor.matmul(
                        ps[:, g, 0:448],
                        lhsT=s1_sb[:, :],
                        rhs=x9_tiles[q][:, 2 * ypl:2 * ypl + 2, :224],
                        start=True, stop=True)
                v = ps[:, :, 0:448].rearrange("p g (r x w) -> p g r x w",
                                              r=2, w=2)
                t1 = apool.tile([128, 2, 2, 112, 2], BF16, tag="act")
                nc.scalar.activation(t1[:, :, :, :, :], v[:, :, :, :, :],
                                     RELU, bias=cb1_sb[:, :])
                m1 = mpool.tile([128, 2, 112, 2], BF16, tag="mx")
                nc.vector.tensor_max(m1[:, :, :, :], t1[:, :, 0, :, :],
                                     t1[:, :, 1, :, :])
                nc.vector.tensor_max(pool1[:, 2 * T:2 * T + 2, :],
                                     m1[:, :, :, 0], m1[:, :, :, 1])

            def repack_chunk(ck):
                r0 = 14 * ck
                for h in range(2):
                    for e in range(2):
                        nc.sync.dma_start(
                            out=c2in[64 * e:64 * e + 64,
                                     56 * h + 1 + r0:56 * h + 15 + r0,
                                     1 - e:113 - e],
                            in_=pool1[64 * h:64 * h + 64, r0:r0 + 14, :])

            def conv2_step(j):
                ps = pspool.tile([128, 2, 512], F32, tag="ps")
                for sub in range(2):
                    y0 = 8 * j + 4 * sub
                    for t in range(6):
                        dy, grp = t // 2, t % 2
                        nc.tensor.matmul(
                            ps[:, sub, 0:448],
                            lhsT=s2_sb[:, t, :],
                            rhs=c2in[:, y0 + dy:y0 + dy + 4,
                                     2 * grp:2 * grp + 112],
                            start=(t == 0), stop=(t == 5))
                v = ps[:, :, 0:448].rearrange("p s (r x w) -> p s r x w",
                                              r=4, w=2)
                t2 = apool.tile([128, 2, 4, 56, 2], BF16, tag="act")
                nc.scalar.activation(t2[:, :, :, :, :], v[:, :, :, :, :],
                                     RELU, bias=cb2_sb[:, :])
                t2v = t2[:, :, :, :, :].rearrange(
                    "p s (q rr) x w -> p s q rr x w", q=2)
                m2 = mpool.tile([128, 2, 2, 56, 2], BF16, tag="mx")
                nc.vector.tensor_max(m2[:, :, :, :, :], t2v[:, :, :, 0, :, :],
                                     t2v[:, :, :, 1, :, :])
                hv = h_sb[:, 224 * j:224 * j + 224].rearrange(
                    "p (s q x) -> p s q x", s=2, q=2)
                nc.vector.tensor_max(hv[:, :, :, :], m2[:, :, :, :, 0],
                                     m2[:, :, :, :, 1])

            # transpose scheduling: blocks grouped into psum tiles of 8
            tp_state = {"tile": None, "n0": 0}

            def emit_transposes(blocks):
                for b in blocks:
                    if tp_state["tile"] is None:
                        tp_state["tile"] = pspool.tile([128, 8, 128], BF16,
                                                       tag="ps", name="tpb")
                        tp_state["n0"] = b
                    tp = tp_state["tile"]
                    nc.tensor.transpose(
                        tp[:, b - tp_state["n0"], :],
                        h_sb[:, 128 * b:128 * b + 128],
                        ident_sb[:, :])
                    if b - tp_state["n0"] == 7 or b == NT - 1:
                        n0, n = tp_state["n0"], b - tp_state["n0"] + 1
                        ov = hTo[:, :].rearrange(
                            "p (d i s j) -> p s i d j", d=8, i=4, s=NT)
                        iv = tp[:, 0:n, :].rearrange(
                            "p s (i d j) -> p s i d j", i=4, d=8)
                        nc.vector.tensor_copy(
                            ov[:, n0:n0 + n, :, :, :], iv[:, :, :, :, :])
                        tp_state["tile"] = None

            if stop_after in ("ps0", "c1t0"):
                ps = pspool.tile([128, 2, 512], F32, tag="ps")
                for g in range(2):
                    nc.tensor.matmul(
                        ps[:, g, 0:448], lhsT=s1_sb[:, :],
                        rhs=x9_tiles[0][:, 2 * g:2 * g + 2, :224],
                        start=True, stop=True)
                dbg = smpool.tile([4, NOUT], F32, tag="outsb")
                nc.vector.tensor_copy(dbg[:, :], ps[0:4, 0, 0:4])
                nc.sync.dma_start(out=out_t[:, :], in_=dbg[:, :])
                return

            # interleave conv1 / repack / conv2 / transposes
            blocks_after = {j: [b for b in range(NT) if _jb(b) == j]
                            for j in range(14)}
            for T in range(7):
                conv1_step(T)
            repack_chunk(0)
            for T in range(7, 14):
                conv1_step(T)
            repack_chunk(1)
            for j in range(3):
                conv2_step(j)
                emit_transposes(blocks_after[j])
            for T in range(14, 21):
                conv1_step(T)
            repack_chunk(2)
            for j in range(3, 5):
                conv2_step(j)
                emit_transposes(blocks_after[j])
            for T in range(21, 28):
                conv1_step(T)
            repack_chunk(3)
            for j in range(5, 14):
                conv2_step(j)
                emit_transposes(blocks_after[j])

            if stop_after == "conv1":
                dbg = smpool.tile([4, NOUT], F32, tag="outsb")
                nc.vector.tensor_copy(dbg[:, :], pool1[0:4, 0, 0:4])
                nc.sync.dma_start(out=out_t[:, :], in_=dbg[:, :])
                return

            if stop_after == "conv2":
                dbg = smpool.tile([4, NOUT], F32, tag="outsb")
                nc.vector.tensor_copy(dbg[:, :], h_sb[0:4, 0:4])
                nc.sync.dma_start(out=out_t[:, :], in_=dbg[:, :])
                return

            # -------- AllToAll: batch-shard -> feature-shard --------
            for d in range(N_CORES):
                nc.sync.dma_start(out=a2a_in[d, :, :],
                                  in_=hTo[:, 400 * d:400 * d + 400])
            nc.gpsimd.collective_compute(
                "AllToAll", mybir.AluOpType.bypass, replica_groups=groups,
                ins=[a2a_in[:, :, :]], outs=[a2a_out[:, :, :]])

            # hT: [p_sp, img32, k=(blk, ch)]
            hT = midpool.tile([128, B, NK], BF16, tag="mid")
            for s in range(N_CORES):
                nc.sync.dma_start(
                    out=hT[:, 4 * s:4 * s + 4, :].rearrange(
                        "p i k -> p (i k)"),
                    in_=a2a_out[s, :, :])

            if stop_after == "a2a":
                dbg = smpool.tile([4, NOUT], F32, tag="outsb")
                nc.vector.tensor_copy(dbg[:, :], hT[0:4, 0:4, 0])
                nc.sync.dma_start(out=out_t[:, :], in_=dbg[:, :])
                return

            # ---------------- fc1 partial ----------------
            fc1_ps = pspool.tile([32, H1], F32, tag="ps")
            for k in range(NK):
                nc.tensor.matmul(fc1_ps[:, :], lhsT=hT[:, :, k],
                                 rhs=wts[k // KPC][:, k % KPC, :],
                                 start=(k == 0), stop=(k == NK - 1))
            fc1_sb = smpool.tile([B, H1], F32, tag="fc1")
            nc.vector.tensor_copy(fc1_sb[:, :], fc1_ps[:, :])
            nc.sync.dma_start(out=rs_in[:, :], in_=fc1_sb[:, :])

            if stop_after == "fc1":
                nc.sync.dma_start(out=out_t[:, :], in_=fc1_sb[0:4, 0:4])
                return

            # -------- ReduceScatter + bias + relu + fc2 --------
            nc.gpsimd.collective_compute(
                "ReduceScatter", mybir.AluOpType.add, replica_groups=groups,
                ins=[rs_in[:, :]], outs=[rs_out[:, :]])

            h2row = smpool.tile([4, H1], F32, tag="h2row")
            nc.sync.dma_start(out=h2row[:, :], in_=rs_out[:, :])
            tp2 = pspool.tile([128, 4, 4], F32, tag="ps")
            for k in range(4):
                nc.tensor.transpose(tp2[:, k, :],
                                    h2row[:, 128 * k:128 * k + 128],
                                    ident4_sb[:, :])
            h2t = smpool.tile([128, 4, 4], F32, tag="h2t")   # [p, k, img]
            nc.vector.tensor_add(h2t[:, :, :], tp2[:, :, :], b1t_sb[:, :, :])
            nc.scalar.activation(h2t[:, :, :], h2t[:, :, :], RELU)

            fc2_ps = pspool.tile([4, 4], F32, tag="ps")
            for k in range(4):
                nc.tensor.matmul(fc2_ps[:, :], lhsT=h2t[:, k, :],
                                 rhs=w2t_sb[:, k, :],
                                 start=(k == 0), stop=(k == 3))
            out_sb = smpool.tile([4, NOUT], F32, tag="outsb")
            nc.vector.tensor_add(out_sb[:, :], fc2_ps[:, :], b2t_sb[:, :])
            nc.sync.dma_start(out=out_t[:, :], in_=out_sb[:, :])


def _get_program(stop_after: str = 'full'):
    key = ("prog", stop_after)
    if key not in _CACHE:
        _CACHE[key] = _build_program(stop_after)
    return _CACHE[key]


def _host_prep(x, conv1_w, conv1_b, conv2_w, conv2_b, values, w_idx1,
               fc1_b, w_idx2, fc2_b):
    """Build per-core input maps (numpy, bf16 for PE-facing tensors)."""
    f32 = np.float32
    x = np.asarray(x, f32)
    conv1_w = np.asarray(conv1_w, f32)
    conv2_w = np.asarray(conv2_w, f32)
    values = np.asarray(values, f32)
    w_idx1 = np.asarray(w_idx1)
    w_idx2 = np.asarray(w_idx2)

    x_pad = np.zeros((B, 226, 232), f32)
    x_pad[:, 1:225, 1:225] = x[:, 0]

    # x9[c]: [72, 112, 232]; partition (dy*3+dx)*8 + h, h = 2*img_loc + half
    x9 = np.zeros((N_CORES, 72, PH, 232), f32)
    for dy in range(3):
        for dx in range(3):
            for h in range(8):
                il, half = h // 2, h % 2
                y0 = PH * half
                for c in range(N_CORES):
                    x9[c, (dy * 3 + dx) * 8 + h, :, :232 - dx] = \
                        x_pad[4 * c + il, y0 + dy:y0 + dy + PH, dx:]

    # conv1 stationary: M-order = half*64 + img*16 + oc
    s1 = np.zeros((72, 128), f32)
    for dy in range(3):
        for dx in range(3):
            for h in range(8):
                il, half = h // 2, h % 2
                m0 = 64 * half + 16 * il
                s1[(dy * 3 + dx) * 8 + h, m0:m0 + C1] = conv1_w[:, 0, dy, dx]

    # conv2 stationaries [6, 128, 128]: pass t = dy*2 + grp;
    # row p = e*64 + img*16 + ic supplies tap dx = 2*grp + e;
    # col q = oc*4 + img (block-diagonal in img)
    s2 = np.zeros((6, 128, 128), f32)
    for t in range(6):
        dy, grp = t // 2, t % 2
        for e in range(2):
            dx = 2 * grp + e
            if dx > 2:
                continue
            for img in range(4):
                for ic in range(C1):
                    s2[t, 64 * e + 16 * img + ic,
                       32 * img:32 * img + C2] = conv2_w[:, ic, dy, dx]

    # fc1 weight shard, padded feature order (p, k=(blk, ch)):
    # feature(p, t, j) = ch j, spatial 128*t + p (zero for pad slots)
    w1ts = []
    for c in range(N_CORES):
        idx = w_idx1[:, FSH * c:FSH * (c + 1)]           # [512, 12544]
        Wsh = values[idx].astype(f32)                    # [512, 12544]
        Wp = np.zeros((H1, 4, SPP), f32)
        Wp[:, :, :SP] = Wsh.reshape(H1, 4, SP)
        Wp = Wp.reshape(H1, 4, NT, 128).transpose(3, 2, 1, 0)  # [128,25,4,512]
        w1ts.append(np.ascontiguousarray(
            Wp.reshape(128, NK, H1)).astype(BF16NP))

    b1t = np.repeat(np.asarray(fc1_b, f32).reshape(4, 128).T[:, :, None],
                    4, axis=2).copy()                    # [128, k4, img4]
    w2t = np.ascontiguousarray(values[w_idx2].T).astype(f32)  # [512, 4]
    b2t = np.broadcast_to(np.asarray(fc2_b, f32), (4, 4)).copy()

    cb1 = np.zeros((128, 1), f32)
    for half in range(2):
        for il in range(4):
            m0 = 64 * half + 16 * il
            cb1[m0:m0 + C1, 0] = np.asarray(conv1_b, f32)
    cb2 = np.zeros((128, 1), f32)
    for img in range(4):
        cb2[32 * img:32 * img + C2, 0] = np.asarray(conv2_b, f32)

    ident = np.eye(128, dtype=f32).astype(BF16NP)

    s1 = s1.astype(BF16NP)
    s2 = s2.astype(BF16NP)
    in_maps = []
    for c in range(N_CORES):
        in_maps.append({
            "x9": np.ascontiguousarray(x9[c]).astype(BF16NP),
            "s1": s1, "s2": s2,
            "w1t": w1ts[c],
            "b1t": b1t, "w2t": w2t, "b2t": b2t,
            "cb1": cb1, "cb2": cb2, "ident": ident,
            "ident4": np.eye(4, dtype=f32),
        })
    return in_maps


def kernel(x, conv1_w, conv1_b, conv2_w, conv2_b, values, w_idx1, fc1_b,
           w_idx2, fc2_b, _trace=False, _trace_kwargs=None,
           _stop_after='full'):
    nc = _get_program(_stop_after)
    in_maps = _host_prep(x, conv1_w, conv1_b, conv2_w, conv2_b, values,
                         w_idx1, fc1_b, w_idx2, fc2_b)
    res = run_bass_kernel_spmd(nc, in_maps, core_ids=list(range(N_CORES)),
                               trace=_trace, **(_trace_kwargs or {}))
    out = np.zeros((B, NOUT), np.float32)
    for c in range(N_CORES):
        out[4 * c:4 * c + 4] = res.results[c]["out"]
    if _trace:
        kernel.last_result = res
    return out


if __name__ == "__main__":
    rng = np.random.default_rng(0)
    ins = {
        "x": rng.standard_normal((B, 1, IMG, IMG), dtype=np.float32),
        "conv1_w": rng.standard_normal((16, 1, 3, 3), dtype=np.float32) * 0.1,
        "conv1_b": np.zeros(16, np.float32),
        "conv2_w": rng.standard_normal((32, 16, 3, 3), dtype=np.float32) * 0.05,
        "conv2_b": np.zeros(32, np.float32),
        "values": np.sort(rng.standard_normal(4096).astype(np.float32) * 0.01),
        "w_idx1": rng.integers(0, 4096, (512, FEAT), dtype=np.int32),
        "fc1_b": np.zeros(512, np.float32),
        "w_idx2": rng.integers(0, 4096, (4, 512), dtype=np.int32),
        "fc2_b": np.zeros(4, np.float32),
    }
    out = kernel(**ins)
    print("out shape", out.shape, "sample row", out[0])
